# revision 7
# baseline (speedup 1.0000x reference)
"""Dense transformer block (single-head causal attention + SiLU FFN) on 8 NeuronCores.

Sharding: data-parallel over batch — each of the 8 cores runs one batch element
end-to-end; no collectives.

Per-core pipeline (T=2048, C=1024, fp32 in HBM, float32r matmuls):
  P0  rmsnorm(x) -> h, PE-transpose -> hT [C,T]
  P1  v = h @ wv (SBUF-resident), qT/kT = (wq|wk)^T h^T -> HBM
  P2  causal attention in S^T layout: S^T = kT^T qT tiles, exp (no max-sub:
      |scores| <= ~2 for this distribution), diagonal tri-mask, o = P @ v and
      l = P @ 1 accumulated on PE, o/l per-row rescale
  P3  o @ w_proj + x -> y; rmsnorm(y) -> h2, transpose -> h2T
  P4  uT = (w1g)^T h2T, silu, f = u @ w2 + y -> out
g1/g2 and the 1/sqrt(C) score scale are folded into the weights on the host.
"""
import sys

sys.path.insert(0, "/opt/trn_rl_repo")

import numpy as np

import concourse.bass as bass
import concourse.mybir as mybir
import concourse.tile as tile
from concourse import bacc
from concourse.bass_utils import run_bass_kernel_spmd
from concourse.masks import make_identity

P = 128
T = 2048
C = 1024
TT = T // P          # 16 row tiles
CT = C // P          # 8 contraction tiles
NQ = T // 512        # 4 query blocks of 512
f32 = mybir.dt.float32
f32r = mybir.dt.float32r
AF = mybir.ActivationFunctionType
Alu = mybir.AluOpType

_NC_CACHE = {}


def _rmsnorm_tiles(nc, pools, x_t, n):
    """ssq/rstd for one [128, C] tile; returns rstd [128,1] fp32 AP."""
    scr, small, eps_t = pools
    scratch = scr.tile([P, n], f32, tag="scratch")
    ssq = small.tile([P, 1], f32, tag="ssq")
    nc.scalar.activation(scratch[:], x_t, AF.Square, accum_out=ssq[:])
    rms = small.tile([P, 1], f32, tag="rms")
    nc.scalar.activation(rms[:], ssq[:], AF.Sqrt, scale=1.0 / n, bias=eps_t[:])
    rstd = small.tile([P, 1], f32, tag="rstd")
    nc.vector.reciprocal(rstd[:], rms[:])
    return rstd


def _transpose_row(nc, trp_pool, ident, src_t, dst, tt):
    """PE-transpose a [128, C] f32r tile into dst[:, ct, tt*128:(tt+1)*128]."""
    nfree = src_t.shape[-1]
    nhalf = nfree // 512
    for half in range(nhalf):
        trp = trp_pool.tile([P, 512], f32r, tag="trp")
        for j in range(4):
            cb = half * 4 + j
            nc.tensor.transpose(
                trp[:, j * P:(j + 1) * P], src_t[:, cb * P:(cb + 1) * P], ident[:]
            )
        nc.vector.tensor_copy(
            dst[:, half * 4:(half + 1) * 4, tt * P:(tt + 1) * P],
            trp[:].rearrange("p (j q) -> p j q", j=4),
        )


def build_nc(sim_silu=False, phases=6):
    nc = bacc.Bacc(None, target_bir_lowering=False)

    x = nc.dram_tensor("x", [T, C], f32, kind="ExternalInput")
    wq = nc.dram_tensor("wq", [C, C], f32, kind="ExternalInput")
    wk = nc.dram_tensor("wk", [C, C], f32, kind="ExternalInput")
    wv = nc.dram_tensor("wv", [C, C], f32, kind="ExternalInput")
    wp = nc.dram_tensor("wp", [C, C], f32, kind="ExternalInput")
    w1g = nc.dram_tensor("w1g", [C, 2 * C], f32, kind="ExternalInput")
    w2 = nc.dram_tensor("w2", [2 * C, C], f32, kind="ExternalInput")
    tri = nc.dram_tensor("tri", [P, P], f32, kind="ExternalInput")
    out = nc.dram_tensor("out", [T, C], f32, kind="ExternalOutput")

    qT_hbm = nc.dram_tensor("qT_hbm", [C, T], f32)
    kT_hbm = nc.dram_tensor("kT_hbm", [C, T], f32)
    o_hbm = nc.dram_tensor("o_hbm", [T, C], f32)
    y_hbm = nc.dram_tensor("y_hbm", [T, C], f32)

    with tile.TileContext(nc) as tc:
        with (
            tc.tile_pool(name="const", bufs=1) as const,
            tc.tile_pool(name="small", bufs=4) as small,
            tc.tile_pool(name="scr", bufs=2) as scr,
        ):
            ident_f = const.tile([P, P], f32)
            make_identity(nc, ident_f)
            ident = const.tile([P, P], f32r)
            nc.vector.tensor_copy(ident[:], ident_f[:])
            ones_f = const.tile([P, 2], f32)
            nc.vector.memset(ones_f, 1.0)
            ones = const.tile([P, 2], f32r)
            nc.vector.tensor_copy(ones[:], ones_f[:])
            eps_t = const.tile([P, 1], f32)
            nc.vector.memset(eps_t, 1e-6)
            tri_sb = const.tile([P, P], f32r)
            nc.sync.dma_start(out=tri_sb, in_=tri[:, :].bitcast(f32r))
            norm_pools = (scr, small, eps_t)

            persist = tc.alloc_tile_pool(name="persist", bufs=1)
            v_sb = persist.tile([P, TT, C], f32r)       # 64KB/part

            # ---------------- P0: norm1 + hT ----------------
            with (
                tc.tile_pool(name="p0", bufs=3) as p0,
                tc.tile_pool(name="hT_pool", bufs=1) as hT_pool,
                tc.tile_pool(name="trp", bufs=2, space="PSUM") as trp_pool,
            ):
                hT = hT_pool.tile([P, CT, T], f32r)     # 64KB/part
                for tt in range(TT):
                    x_t = p0.tile([P, C], f32, tag="x_t")
                    nc.sync.dma_start(out=x_t, in_=x[tt * P:(tt + 1) * P, :])
                    rstd = _rmsnorm_tiles(nc, norm_pools, x_t[:], C)
                    h_t = p0.tile([P, C], f32r, tag="h_t")
                    nc.vector.tensor_scalar_mul(h_t[:], x_t[:], rstd[:])
                    _transpose_row(nc, trp_pool, ident, h_t[:], hT[:], tt)

                # ---------------- P1a: v ----------------
                if phases >= 2:
                 with (
                    tc.tile_pool(name="wv_pool", bufs=1) as wv_pool,
                    tc.tile_pool(name="mm", bufs=4, space="PSUM") as mm_pool,
                ):
                    wv_sb = wv_pool.tile([P, CT, C], f32r)
                    nc.sync.dma_start(
                        out=wv_sb,
                        in_=wv[:, :].rearrange("(ct p) n -> p ct n", p=P).bitcast(f32r),
                    )
                    for tt in range(TT):
                        for ch in range(2):
                            psum = mm_pool.tile([P, 512], f32, tag="mm")
                            for ct in range(CT):
                                nc.tensor.matmul(
                                    psum[:],
                                    hT[:, ct, tt * P:(tt + 1) * P],
                                    wv_sb[:, ct, ch * 512:(ch + 1) * 512],
                                    start=(ct == 0), stop=(ct == CT - 1),
                                )
                            nc.vector.tensor_copy(
                                v_sb[:, tt, ch * 512:(ch + 1) * 512], psum[:]
                            )

                # ---------------- P1b: qT, kT -> HBM ----------------
                if phases >= 3:
                 with (
                    tc.tile_pool(name="wstream", bufs=3) as wstream,
                    tc.tile_pool(name="stage", bufs=4) as stage_pool,
                    tc.tile_pool(name="mmq", bufs=4, space="PSUM") as mmq_pool,
                ):
                    for w_src, dst in ((wq, qT_hbm), (wk, kT_hbm)):
                        for m in range(CT):
                            wsl = wstream.tile([P, CT, P], f32r, tag="wsl")
                            nc.sync.dma_start(
                                out=wsl,
                                in_=w_src[:, m * P:(m + 1) * P]
                                .rearrange("(ct p) m -> p ct m", p=P)
                                .bitcast(f32r),
                            )
                            for n in range(4):
                                psum = mmq_pool.tile([P, 512], f32, tag="mmq")
                                for ct in range(CT):
                                    nc.tensor.matmul(
                                        psum[:],
                                        wsl[:, ct, :],
                                        hT[:, ct, n * 512:(n + 1) * 512],
                                        start=(ct == 0), stop=(ct == CT - 1),
                                    )
                                stg = stage_pool.tile([P, 512], f32, tag="stg")
                                nc.vector.tensor_copy(stg[:], psum[:])
                                nc.sync.dma_start(
                                    out=dst[m * P:(m + 1) * P, n * 512:(n + 1) * 512],
                                    in_=stg[:],
                                )

            # ---------------- P2: attention ----------------
            if phases >= 4:
             with (
                tc.tile_pool(name="kT_pool", bufs=1) as kT_pool,
                tc.tile_pool(name="qT_pool", bufs=1) as qT_pool,
                tc.tile_pool(name="exp_pool", bufs=16) as exp_pool,
                tc.tile_pool(name="ost", bufs=2) as ost_pool,
                tc.tile_pool(name="stp", bufs=2, space="PSUM") as stp_pool,
                tc.tile_pool(name="ops", bufs=4, space="PSUM") as ops_pool,
                tc.tile_pool(name="lps", bufs=1, space="PSUM") as lps_pool,
            ):
                kT_sb = kT_pool.tile([P, CT, T], f32r)
                for j in range(NQ):
                    nc.sync.dma_start(
                        out=kT_sb[:, :, j * 512:(j + 1) * 512],
                        in_=kT_hbm[:, j * 512:(j + 1) * 512]
                        .rearrange("(ct p) t -> p ct t", p=P)
                        .bitcast(f32r),
                    )
                    qT_j = qT_pool.tile([P, CT, 512], f32r, tag="qTj")
                    nc.sync.dma_start(
                        out=qT_j,
                        in_=qT_hbm[:, j * 512:(j + 1) * 512]
                        .rearrange("(ct p) t -> p ct t", p=P)
                        .bitcast(f32r),
                    )
                    nkt = 4 * (j + 1)
                    expst = []
                    for kt in range(nkt):
                        stp = stp_pool.tile([P, 512], f32, tag="stp")
                        for ct in range(CT):
                            nc.tensor.matmul(
                                stp[:],
                                kT_sb[:, ct, kt * P:(kt + 1) * P],
                                qT_j[:, ct, :],
                                start=(ct == 0), stop=(ct == CT - 1),
                            )
                        e_t = exp_pool.tile([P, 512], f32r, tag="expst")
                        nc.scalar.activation(e_t[:], stp[:], AF.Exp)
                        if kt >= 4 * j:
                            s = kt - 4 * j
                            nc.vector.tensor_mul(
                                e_t[:, s * P:(s + 1) * P],
                                e_t[:, s * P:(s + 1) * P],
                                tri_sb[:],
                            )
                        expst.append(e_t)
                    l_ps = lps_pool.tile([P, 8], f32, tag="lps")
                    for qsub in range(4):
                        qt = 4 * j + qsub
                        o_ps = [
                            ops_pool.tile([P, 512], f32, tag="ops", name=f"ops{i}")
                            for i in range(2)
                        ]
                        for kt in range(qt + 1):
                            lhs = expst[kt][:, qsub * P:(qsub + 1) * P]
                            st = (kt == 0)
                            sp = (kt == qt)
                            nc.tensor.matmul(
                                o_ps[0][:], lhs, v_sb[:, kt, 0:512], start=st, stop=sp
                            )
                            nc.tensor.matmul(
                                o_ps[1][:], lhs, v_sb[:, kt, 512:1024], start=st, stop=sp
                            )
                            nc.tensor.matmul(
                                l_ps[:, 2 * qsub:2 * qsub + 2], lhs, ones[:],
                                start=st, stop=sp,
                            )
                        l_sb = small.tile([P, 1], f32, tag="l_sb")
                        nc.vector.tensor_copy(l_sb[:], l_ps[:, 2 * qsub:2 * qsub + 1])
                        rl = small.tile([P, 1], f32, tag="rl")
                        nc.vector.reciprocal(rl[:], l_sb[:])
                        o_st = ost_pool.tile([P, C], f32, tag="o_st")
                        nc.vector.tensor_scalar_mul(o_st[:, 0:512], o_ps[0][:], rl[:])
                        nc.vector.tensor_scalar_mul(o_st[:, 512:1024], o_ps[1][:], rl[:])
                        nc.sync.dma_start(
                            out=o_hbm[qt * P:(qt + 1) * P, :], in_=o_st[:]
                        )

            persist.release()

            # ---------------- P3: proj + norm2 + h2T ----------------
            if phases >= 5:
             with (
                tc.tile_pool(name="h2T_pool", bufs=1) as h2T_pool,
            ):
                h2T = h2T_pool.tile([P, CT, T], f32r)
                with (
                    tc.tile_pool(name="wp_pool", bufs=1) as wp_pool,
                    tc.tile_pool(name="p3", bufs=2) as p3,
                    tc.tile_pool(name="oT_pool", bufs=2) as oT_pool,
                    tc.tile_pool(name="trp3", bufs=2, space="PSUM") as trp3_pool,
                    tc.tile_pool(name="mm3", bufs=4, space="PSUM") as mm3_pool,
                ):
                    wp_sb = wp_pool.tile([P, CT, C], f32r)
                    nc.sync.dma_start(
                        out=wp_sb,
                        in_=wp[:, :].rearrange("(ct p) n -> p ct n", p=P).bitcast(f32r),
                    )
                    for tt in range(TT):
                        o_t = p3.tile([P, C], f32r, tag="o_t")
                        nc.sync.dma_start(
                            out=o_t, in_=o_hbm[tt * P:(tt + 1) * P, :].bitcast(f32r)
                        )
                        oT_t = oT_pool.tile([P, CT, P], f32r, tag="oT")
                        _transpose_row(nc, trp3_pool, ident, o_t[:], oT_t[:], 0)
                        x_t = p3.tile([P, C], f32, tag="x_t3")
                        nc.sync.dma_start(out=x_t, in_=x[tt * P:(tt + 1) * P, :])
                        y_t = p3.tile([P, C], f32, tag="y_t")
                        for ch in range(2):
                            psum = mm3_pool.tile([P, 512], f32, tag="mm3")
                            for ct in range(CT):
                                nc.tensor.matmul(
                                    psum[:],
                                    oT_t[:, ct, :],
                                    wp_sb[:, ct, ch * 512:(ch + 1) * 512],
                                    start=(ct == 0), stop=(ct == CT - 1),
                                )
                            nc.vector.tensor_add(
                                y_t[:, ch * 512:(ch + 1) * 512],
                                psum[:],
                                x_t[:, ch * 512:(ch + 1) * 512],
                            )
                        nc.sync.dma_start(
                            out=y_hbm[tt * P:(tt + 1) * P, :], in_=y_t[:]
                        )
                        rstd2 = _rmsnorm_tiles(nc, norm_pools, y_t[:], C)
                        h2_t = p3.tile([P, C], f32r, tag="h2_t")
                        nc.vector.tensor_scalar_mul(h2_t[:], y_t[:], rstd2[:])
                        _transpose_row(nc, trp3_pool, ident, h2_t[:], h2T[:], tt)

                # ---------------- P4: FFN ----------------
                if phases >= 6:
                 with (
                    tc.tile_pool(name="w2_pool", bufs=1) as w2_pool,
                    tc.tile_pool(name="w1s", bufs=3) as w1s_pool,
                    tc.tile_pool(name="u_pool", bufs=16) as u_pool,
                    tc.tile_pool(name="p4", bufs=2) as p4,
                    tc.tile_pool(name="mmu", bufs=2, space="PSUM") as mmu_pool,
                    tc.tile_pool(name="mmf", bufs=4, space="PSUM") as mmf_pool,
                ):
                    w2_sb = w2_pool.tile([P, 2 * CT, C], f32r)
                    nc.sync.dma_start(
                        out=w2_sb,
                        in_=w2[:, :].rearrange("(ct p) n -> p ct n", p=P).bitcast(f32r),
                    )
                    for n in range(4):
                        u_tiles = []
                        for m in range(2 * CT):
                            w1sl = w1s_pool.tile([P, CT, P], f32r, tag="w1sl")
                            nc.sync.dma_start(
                                out=w1sl,
                                in_=w1g[:, m * P:(m + 1) * P]
                                .rearrange("(ct p) m -> p ct m", p=P)
                                .bitcast(f32r),
                            )
                            psum = mmu_pool.tile([P, 512], f32, tag="mmu")
                            for ct in range(CT):
                                nc.tensor.matmul(
                                    psum[:],
                                    w1sl[:, ct, :],
                                    h2T[:, ct, n * 512:(n + 1) * 512],
                                    start=(ct == 0), stop=(ct == CT - 1),
                                )
                            u_t = u_pool.tile([P, 512], f32r, tag="u")
                            if sim_silu:
                                sg = w1s_pool.tile([P, 512], f32, tag="sg")
                                nc.scalar.activation(sg[:], psum[:], AF.Sigmoid)
                                nc.vector.tensor_mul(u_t[:], psum[:], sg[:])
                            else:
                                nc.scalar.activation(u_t[:], psum[:], AF.Silu)
                            u_tiles.append(u_t)
                        for ts in range(4):
                            tt = 4 * n + ts
                            y_t = p4.tile([P, C], f32, tag="y_t4")
                            nc.sync.dma_start(
                                out=y_t, in_=y_hbm[tt * P:(tt + 1) * P, :]
                            )
                            out_t = p4.tile([P, C], f32, tag="out_t")
                            for ch in range(2):
                                psum2 = mmf_pool.tile([P, 512], f32, tag="mmf")
                                for m in range(2 * CT):
                                    nc.tensor.matmul(
                                        psum2[:],
                                        u_tiles[m][:, ts * P:(ts + 1) * P],
                                        w2_sb[:, m, ch * 512:(ch + 1) * 512],
                                        start=(m == 0), stop=(m == 2 * CT - 1),
                                    )
                                nc.vector.tensor_add(
                                    out_t[:, ch * 512:(ch + 1) * 512],
                                    psum2[:],
                                    y_t[:, ch * 512:(ch + 1) * 512],
                                )
                            nc.sync.dma_start(
                                out=out[tt * P:(tt + 1) * P, :], in_=out_t[:]
                            )

            if phases < 6:
                dbg_src = {1: x, 2: x, 3: x, 4: o_hbm, 5: y_hbm}[phases]
                with tc.tile_pool(name="dbg", bufs=2) as dbg:
                    for tt in range(TT):
                        d_t = dbg.tile([P, C], f32, tag="d_t")
                        nc.sync.dma_start(out=d_t, in_=dbg_src[tt * P:(tt + 1) * P, :])
                        nc.sync.dma_start(out=out[tt * P:(tt + 1) * P, :], in_=d_t[:])

    nc.compile()
    return nc


def kernel(x, w_qkv, w_proj, w1, w2, g1, g2):
    x = np.ascontiguousarray(np.asarray(x, dtype=np.float32))
    w_qkv = np.asarray(w_qkv, dtype=np.float32)
    g1 = np.asarray(g1, dtype=np.float32)
    g2 = np.asarray(g2, dtype=np.float32)
    wg = w_qkv * g1[:, None]
    wq_np = np.ascontiguousarray(wg[:, 0:C] * (1.0 / np.sqrt(C)))
    wk_np = np.ascontiguousarray(wg[:, C:2 * C])
    wv_np = np.ascontiguousarray(wg[:, 2 * C:3 * C])
    wp_np = np.ascontiguousarray(np.asarray(w_proj, dtype=np.float32))
    w1g_np = np.ascontiguousarray(np.asarray(w1, dtype=np.float32) * g2[:, None])
    w2_np = np.ascontiguousarray(np.asarray(w2, dtype=np.float32))
    tri_np = np.triu(np.ones((P, P), dtype=np.float32))  # keep k<=q

    if "nc" not in _NC_CACHE:
        _NC_CACHE["nc"] = build_nc()
    nc = _NC_CACHE["nc"]

    B = x.shape[0]
    shared = {
        "wq": wq_np, "wk": wk_np, "wv": wv_np, "wp": wp_np,
        "w1g": w1g_np, "w2": w2_np, "tri": tri_np,
    }
    in_maps = [dict(shared, x=x[b]) for b in range(B)]
    r = run_bass_kernel_spmd(nc, in_maps, list(range(B)))
    return np.stack([r.results[b]["out"] for b in range(B)], axis=0)


if __name__ == "__main__":
    rng = np.random.default_rng(0)
    inputs = {
        "x": rng.standard_normal((8, T, C), dtype=np.float32),
        "w_qkv": (rng.random((C, 3 * C), dtype=np.float32) - 0.5) / 16.0,
        "w_proj": (rng.random((C, C), dtype=np.float32) - 0.5) / 16.0,
        "w1": (rng.random((C, 2 * C), dtype=np.float32) - 0.5) / 16.0,
        "w2": (rng.random((2 * C, C), dtype=np.float32) - 0.5) / 22.6,
        "g1": np.ones(C, dtype=np.float32),
        "g2": np.ones(C, dtype=np.float32),
    }
    out = kernel(**inputs)
    print("out", out.shape, out.dtype, np.abs(out).mean())
